# revision 1
# baseline (speedup 1.0000x reference)
"""GAT (2-layer, PyG-style) on 8 Trainium2 NeuronCores.

Strategy: destination-node sharding (graph parallel). Each core owns a
contiguous range of 6272 destination nodes and all edges pointing into
them (sorted by dst). Source-node features are fetched on-device with
batched indexed gathers (dma_gather) from a per-core *rotated* copy of
the node table, so that a core's own dst rows always sit at small row
indices (dma_gather indices are int16, hence also the A/B table-half
split for the random source indices).

Layer-1 messages are computed on the fly: gather x[src] (bf16, 256 B
rows), then h = x@W1 / e_src = x@w_src / e_dst = x@w_dst as PE matmuls
per 128-edge chunk; segment softmax + scatter-add are done with a
one-hot matmul (S_T^T @ V) accumulated in PSUM per 128-dst-node tile.
The tile tail normalizes by the softmax denominator, applies ReLU and
immediately computes the layer-2 node table row [h2 | e2_src | e2_dst]
via W2. A second launch runs the (structurally identical, 1-head)
layer-2 attention over the host-assembled h2 table and finishes with
log_softmax.
"""

import numpy as np
import ml_dtypes
from contextlib import ExitStack

import concourse.bass as bass
import concourse.mybir as mybir
import concourse.tile as tile
from concourse import bacc
from concourse.bass_utils import run_bass_kernel_spmd

F32 = mybir.dt.float32
BF16 = mybir.dt.bfloat16
I16 = mybir.dt.int16
AF = mybir.ActivationFunctionType
OP = mybir.AluOpType

N = 50000
E = 500000
IN = 128
HID = 64
HEADS = 8
OUT = 40
NEG = 0.2
NCORE = 8
P = 128
TILES = 49
SHARD = TILES * P          # 6272
NPAD = NCORE * SHARD       # 50176
SPLIT = 32768              # int16 table-half split
NB = NPAD - SPLIT          # 17408

_bf16 = ml_dtypes.bfloat16

_CACHE = {}

# Gather sizing: one dma_gather of n indices emits n/16+2 descriptors per
# SDMA engine; with single_packet=True a packet holds at most 64
# descriptors, so calls >992 indices wedge the device. 896 indices
# (58 descs) stays under the cap; multi-call concurrency at this size is
# throttled safely by ucode (verified on HW).
GCAP = 896
GSP = True  # single_packet


def _gather(nc, out3, in_ap, idx_sb, col0, n, elem):
    """dma_gather split into <=GCAP-index calls. out3: [P, 1|chunks, *]
    destination AP covering exactly n indices starting at its origin."""
    done = 0
    while done < n:
        take = min(GCAP, n - done)
        if out3.ndim == 3 and out3.shape[2] != elem:  # transpose=True layout
            o = out3[:, :, done : done + take]
            tr = True
        else:  # [P, chunks, elem] layout
            o = out3[:, done // P : (done + take) // P, :]
            tr = False
        nc.gpsimd.dma_gather(
            out_ap=o,
            in_ap=in_ap,
            idxs_ap=idx_sb[:, col0 + done // 16 : col0 + (done + take) // 16],
            num_idxs=take,
            num_idxs_reg=take,
            elem_size=elem,
            transpose=tr,
            single_packet=GSP,
        )
        done += take


def _wrap16(v):
    """dma_gather index layout: idx[p, j] = stream[j*16 + p%16], replicated
    to 128 partitions."""
    assert len(v) % 16 == 0
    w = v.reshape(-1, 16).T.astype(np.int16)   # [16, n/16]
    return np.tile(w, (8, 1))                  # [128, n/16]


def _prep_edges(edge_index):
    """Bucket edges (+self-loops) by dst core, sort by dst, split by
    src-table half, pad to SPMD-uniform per-tile sizes.

    Returns per-tile padded sizes EA/EB (shared by all cores) and the
    per-core index/metadata streams."""
    src = np.concatenate([np.asarray(edge_index[0]), np.arange(N)]).astype(np.int64)
    dst = np.concatenate([np.asarray(edge_index[1]), np.arange(N)]).astype(np.int64)
    core = dst // SHARD

    pc = []  # per-core (tile -> (a_idx, b_idx, dloc_a, dloc_b))
    nA = np.zeros((NCORE, TILES), np.int64)
    nB = np.zeros((NCORE, TILES), np.int64)
    for c in range(NCORE):
        m = core == c
        s = src[m]
        dl = dst[m] - c * SHARD
        o = np.argsort(dl, kind="stable")
        s = s[o]
        dl = dl[o]
        sr = (s - c * SHARD) % NPAD  # rotated source row
        bounds = np.searchsorted(dl, np.arange(TILES + 1) * P)
        tl = []
        for t in range(TILES):
            lo, hi = bounds[t], bounds[t + 1]
            srt, dlt = sr[lo:hi], dl[lo:hi] % P
            ma = srt < SPLIT
            tl.append((srt[ma], srt[~ma] - SPLIT, dlt[ma], dlt[~ma]))
            nA[c, t] = ma.sum()
            nB[c, t] = (~ma).sum()
        pc.append(tl)

    rup = lambda n: int(-(-n // P) * P)
    EA = [rup(nA[:, t].max()) for t in range(TILES)]
    EB = [rup(nB[:, t].max()) for t in range(TILES)]

    streams = []
    for c in range(NCORE):
        ia, ib, idd, dlc = [], [], [], []
        for t in range(TILES):
            a, b, da, db = pc[c][t]
            pa = np.zeros(EA[t], np.int64)
            pa[: len(a)] = a
            pb = np.zeros(EB[t], np.int64)
            pb[: len(b)] = b
            ia.append(pa)
            ib.append(pb)
            # dst-row gather stream + dst-local values, in slot order [A|B]
            dr = np.zeros(EA[t] + EB[t], np.int64)
            dv = np.full(EA[t] + EB[t], 200.0, np.float32)
            dr[: len(a)] = da + t * P
            dv[: len(a)] = da
            dr[EA[t] : EA[t] + len(b)] = db + t * P
            dv[EA[t] : EA[t] + len(b)] = db
            idd.append(dr)
            dlc.append(dv)
        ept = np.concatenate(idd)
        # L2 combined stream: per tile [srcA-padded | dst] (both read htA)
        iad = [np.concatenate([ia[t], idd[t]]) for t in range(TILES)]
        streams.append(
            dict(
                idxA=_wrap16(np.concatenate(ia)),
                idxB=_wrap16(np.concatenate(ib)),
                idxD=_wrap16(ept),
                idxAD=_wrap16(np.concatenate(iad)),
                dloc=np.concatenate(dlc).reshape(-1, P).T.copy(),  # [128, nchunks]
            )
        )
    return EA, EB, streams


def _build_l1(EA, EB):
    colsA = sum(EA) // 16
    colsB = sum(EB) // 16
    EPT = [a + b for a, b in zip(EA, EB)]
    colsD = sum(EPT) // 16
    nch_tot = sum(EPT) // P

    nc = bacc.Bacc("TRN2", target_bir_lowering=False, debug=False, num_devices=NCORE)
    xtA = nc.dram_tensor("xtA", [SPLIT, IN], BF16, kind="ExternalInput")
    xtB = nc.dram_tensor("xtB", [NB, IN], BF16, kind="ExternalInput")
    idxA = nc.dram_tensor("idxA", [P, max(colsA, 1)], I16, kind="ExternalInput")
    idxB = nc.dram_tensor("idxB", [P, max(colsB, 1)], I16, kind="ExternalInput")
    idxD = nc.dram_tensor("idxD", [P, colsD], I16, kind="ExternalInput")
    dloc = nc.dram_tensor("dloc", [P, nch_tot], F32, kind="ExternalInput")
    w1 = nc.dram_tensor("w1", [P, HEADS * HID], BF16, kind="ExternalInput")
    wsd = nc.dram_tensor("wsd", [P, 2 * HEADS], BF16, kind="ExternalInput")
    w2c = nc.dram_tensor("w2c", [P, 4 * 42], BF16, kind="ExternalInput")
    iot = nc.dram_tensor("iot", [P, P], BF16, kind="ExternalInput")
    idn = nc.dram_tensor("idn", [P, P], BF16, kind="ExternalInput")
    h2row = nc.dram_tensor("h2row", [SHARD, 64], F32, kind="ExternalOutput")

    with tile.TileContext(nc) as tc, ExitStack() as ctx:
        cp = ctx.enter_context(tc.tile_pool(name="const", bufs=1))
        gp = ctx.enter_context(tc.tile_pool(name="gath", bufs=12))
        sp = ctx.enter_context(tc.tile_pool(name="small", bufs=12))
        vp = ctx.enter_context(tc.tile_pool(name="vals", bufs=8))
        rp = ctx.enter_context(tc.tile_pool(name="tail", bufs=3))
        ph = ctx.enter_context(tc.tile_pool(name="ph", bufs=2, space="PSUM"))
        pe = ctx.enter_context(tc.tile_pool(name="pe", bufs=2, space="PSUM"))
        po = ctx.enter_context(tc.tile_pool(name="po", bufs=1, space="PSUM"))
        pz = ctx.enter_context(tc.tile_pool(name="pz", bufs=1, space="PSUM"))
        pt = ctx.enter_context(tc.tile_pool(name="pt", bufs=1, space="PSUM"))
        p2 = ctx.enter_context(tc.tile_pool(name="p2", bufs=1, space="PSUM"))

        w1sb = cp.tile([P, HEADS * HID], BF16)
        nc.sync.dma_start(w1sb[:], w1.ap())
        wsdsb = cp.tile([P, 2 * HEADS], BF16)
        nc.sync.dma_start(wsdsb[:], wsd.ap())
        w2csb = cp.tile([P, 4 * 42], BF16)
        nc.sync.dma_start(w2csb[:], w2c.ap())
        iosb = cp.tile([P, P], BF16)
        nc.sync.dma_start(iosb[:], iot.ap())
        idsb = cp.tile([P, P], BF16)
        nc.sync.dma_start(idsb[:], idn.ap())
        iAsb = cp.tile([P, max(colsA, 1)], I16)
        nc.sync.dma_start(iAsb[:], idxA.ap())
        iBsb = cp.tile([P, max(colsB, 1)], I16)
        nc.sync.dma_start(iBsb[:], idxB.ap())
        iDsb = cp.tile([P, colsD], I16)
        nc.sync.dma_start(iDsb[:], idxD.ap())
        dlsb = cp.tile([P, nch_tot], F32)
        nc.sync.dma_start(dlsb[:], dloc.ap())

        oa = ob = od = co = 0
        for t in range(TILES):
            ea, eb = EA[t], EB[t]
            ept = ea + eb
            nchk = ept // P
            xg = gp.tile([P, 1, ept], BF16, tag="xg")
            if ea:
                _gather(nc, xg[:, :, 0:ea], xtA.ap(), iAsb, oa, ea, IN)
            if eb:
                _gather(nc, xg[:, :, ea:ept], xtB.ap(), iBsb, ob, eb, IN)
            xd = gp.tile([P, 1, ept], BF16, tag="xd")
            _gather(nc, xd[:, :, 0:ept], xtA.ap(), iDsb, od, ept, IN)

            o1ps = po.tile([P, HEADS * HID], F32, tag="o1")
            zps = pz.tile([P, HEADS], F32, tag="z")
            for k in range(nchk):
                ls = xg[:, 0, k * P : (k + 1) * P]
                ld = xd[:, 0, k * P : (k + 1) * P]
                hps = ph.tile([P, HEADS * HID], F32, tag="h")
                nc.tensor.matmul(hps[:], lhsT=ls, rhs=w1sb[:], start=True, stop=True)
                eps = pe.tile([P, HEADS], F32, tag="e")
                nc.tensor.matmul(
                    eps[:], lhsT=ls, rhs=wsdsb[:, 0:HEADS],
                    start=True, stop=False,
                )
                nc.tensor.matmul(
                    eps[:], lhsT=ld,
                    rhs=wsdsb[:, HEADS : 2 * HEADS], start=False, stop=True,
                )
                st = sp.tile([P, P], BF16, tag="st")
                nc.vector.tensor_scalar(
                    out=st[:], in0=iosb[:], scalar1=dlsb[:, co + k : co + k + 1],
                    scalar2=None, op0=OP.is_equal,
                )
                ll = sp.tile([P, HEADS], F32, tag="ll")
                nc.vector.tensor_scalar(
                    out=ll[:], in0=eps[:], scalar1=NEG, scalar2=None, op0=OP.mult
                )
                lr = sp.tile([P, HEADS], F32, tag="lr")
                nc.vector.tensor_tensor(out=lr[:], in0=eps[:], in1=ll[:], op=OP.max)
                p32 = sp.tile([P, HEADS], F32, tag="p32")
                nc.scalar.activation(out=p32[:], in_=lr[:], func=AF.Exp)
                pbf = sp.tile([P, HEADS], BF16, tag="pbf")
                nc.vector.tensor_copy(out=pbf[:], in_=p32[:])
                vt = vp.tile([P, HEADS, HID], BF16, tag="vt")
                nc.vector.tensor_tensor(
                    out=vt[:],
                    in0=hps[:].rearrange("p (h c) -> p h c", c=HID),
                    in1=p32[:].unsqueeze(2).to_broadcast([P, HEADS, HID]),
                    op=OP.mult,
                )
                nc.tensor.matmul(
                    o1ps[:], lhsT=st[:], rhs=vt[:].rearrange("p h c -> p (h c)"),
                    start=(k == 0), stop=(k == nchk - 1),
                )
                nc.tensor.matmul(
                    zps[:], lhsT=st[:], rhs=pbf[:],
                    start=(k == 0), stop=(k == nchk - 1),
                )

            zr = sp.tile([P, HEADS], F32, tag="zr")
            nc.vector.reciprocal(zr[:], zps[:])
            r1 = rp.tile([P, HEADS * HID], BF16, tag="r1")
            for h in range(HEADS):
                nc.scalar.activation(
                    out=r1[:, h * HID : (h + 1) * HID],
                    in_=o1ps[:, h * HID : (h + 1) * HID],
                    func=AF.Relu,
                    scale=zr[:, h : h + 1],
                )
            h2ps = p2.tile([P, 48], F32, tag="h2")
            for j in range(4):
                tp = pt.tile([P, P], BF16, tag="tp")
                nc.tensor.transpose(tp[:], r1[:, j * P : (j + 1) * P], idsb[:])
                tsb = rp.tile([P, P], BF16, tag="tsb")
                nc.scalar.activation(out=tsb[:], in_=tp[:], func=AF.Copy)
                nc.tensor.matmul(
                    h2ps[:, 0:42], lhsT=tsb[:], rhs=w2csb[:, j * 42 : (j + 1) * 42],
                    start=(j == 0), stop=(j == 3),
                )
            o1 = rp.tile([P, 64], F32, tag="o1s")
            nc.scalar.activation(out=o1[:, 0:42], in_=h2ps[:, 0:42], func=AF.Copy)
            nc.vector.memset(o1[:, 42:64], 0.0)
            nc.sync.dma_start(h2row.ap()[t * P : (t + 1) * P, :], o1[:])

            oa += ea // 16
            ob += eb // 16
            od += ept // 16
            co += nchk
    nc.compile()
    return nc


def _build_l2(EA, EB):
    colsA = sum(EA) // 16
    colsB = sum(EB) // 16
    EPT = [a + b for a, b in zip(EA, EB)]
    colsD = sum(EPT) // 16
    nch_tot = sum(EPT) // P

    colsAD = colsA + colsD
    nc = bacc.Bacc("TRN2", target_bir_lowering=False, debug=False, num_devices=NCORE)
    htA = nc.dram_tensor("htA", [SPLIT, 64], F32, kind="ExternalInput")
    htB = nc.dram_tensor("htB", [NB, 64], F32, kind="ExternalInput")
    idxAD = nc.dram_tensor("idxAD", [P, colsAD], I16, kind="ExternalInput")
    idxB = nc.dram_tensor("idxB", [P, max(colsB, 1)], I16, kind="ExternalInput")
    dloc = nc.dram_tensor("dloc", [P, nch_tot], F32, kind="ExternalInput")
    iot = nc.dram_tensor("iot", [P, P], BF16, kind="ExternalInput")
    out2 = nc.dram_tensor("out2", [SHARD, OUT], F32, kind="ExternalOutput")

    with tile.TileContext(nc) as tc, ExitStack() as ctx:
        cp = ctx.enter_context(tc.tile_pool(name="const", bufs=1))
        gp = ctx.enter_context(tc.tile_pool(name="gath", bufs=3))
        sp = ctx.enter_context(tc.tile_pool(name="small", bufs=4))
        rp = ctx.enter_context(tc.tile_pool(name="tail", bufs=2))
        po = ctx.enter_context(tc.tile_pool(name="po", bufs=2, space="PSUM"))
        pz = ctx.enter_context(tc.tile_pool(name="pz", bufs=2, space="PSUM"))

        iosb = cp.tile([P, P], BF16)
        nc.sync.dma_start(iosb[:], iot.ap())
        iADsb = cp.tile([P, colsAD], I16)
        nc.sync.dma_start(iADsb[:], idxAD.ap())
        iBsb = cp.tile([P, max(colsB, 1)], I16)
        nc.sync.dma_start(iBsb[:], idxB.ap())
        dlsb = cp.tile([P, nch_tot], F32)
        nc.sync.dma_start(dlsb[:], dloc.ap())

        oad = ob = co = 0
        for t in range(TILES):
            ea, eb = EA[t], EB[t]
            ept = ea + eb
            nchk = ept // P
            # tile layout: [srcA chunks | dst chunks | srcB chunks]
            gad = gp.tile([P, nchk + ept // P, 64], F32, tag="g2")
            _gather(nc, gad[:, 0 : (ea + ept) // P, :], htA.ap(), iADsb, oad,
                    ea + ept, 64)
            if eb:
                _gather(nc, gad[:, (ea + ept) // P :, :], htB.ap(), iBsb, ob,
                        eb, 64)

            def _src(k, ea=ea, ept=ept, gad=gad):
                return gad[:, k, :] if k < ea // P else gad[:, ept // P + k, :]

            def _dst(k, ea=ea, gad=gad):
                return gad[:, ea // P + k, :]

            o2ps = po.tile([P, 48], F32, tag="o2")
            z2ps = pz.tile([P, 8], F32, tag="z2")
            for k in range(nchk):
                st = sp.tile([P, P], BF16, tag="st")
                nc.vector.tensor_scalar(
                    out=st[:], in0=iosb[:], scalar1=dlsb[:, co + k : co + k + 1],
                    scalar2=None, op0=OP.is_equal,
                )
                lg = sp.tile([P, 1], F32, tag="lg")
                nc.vector.tensor_tensor(
                    out=lg[:], in0=_src(k)[:, 40:41], in1=_dst(k)[:, 41:42],
                    op=OP.add,
                )
                ll = sp.tile([P, 1], F32, tag="ll")
                nc.vector.tensor_scalar(
                    out=ll[:], in0=lg[:], scalar1=NEG, scalar2=None, op0=OP.mult
                )
                lr = sp.tile([P, 1], F32, tag="lr")
                nc.vector.tensor_tensor(out=lr[:], in0=lg[:], in1=ll[:], op=OP.max)
                p32 = sp.tile([P, 1], F32, tag="p32")
                nc.scalar.activation(out=p32[:], in_=lr[:], func=AF.Exp)
                pbf = sp.tile([P, 1], BF16, tag="pbf")
                nc.vector.tensor_copy(out=pbf[:], in_=p32[:])
                v2 = sp.tile([P, OUT], BF16, tag="v2")
                nc.scalar.activation(
                    out=v2[:], in_=_src(k)[:, 0:OUT], func=AF.Identity,
                    scale=p32[:],
                )
                nc.tensor.matmul(
                    o2ps[:, 0:OUT], lhsT=st[:], rhs=v2[:],
                    start=(k == 0), stop=(k == nchk - 1),
                )
                nc.tensor.matmul(
                    z2ps[:, 0:1], lhsT=st[:], rhs=pbf[:],
                    start=(k == 0), stop=(k == nchk - 1),
                )

            zr = sp.tile([P, 1], F32, tag="zr")
            nc.vector.reciprocal(zr[:], z2ps[:, 0:1])
            av = rp.tile([P, OUT], F32, tag="av")
            nc.vector.tensor_scalar(
                out=av[:], in0=o2ps[:, 0:OUT], scalar1=zr[:], scalar2=None,
                op0=OP.mult,
            )
            mx = sp.tile([P, 1], F32, tag="mx")
            nc.vector.reduce_max(out=mx[:], in_=av[:], axis=mybir.AxisListType.X)
            tm = rp.tile([P, OUT], F32, tag="tm")
            nc.vector.tensor_scalar(
                out=tm[:], in0=av[:], scalar1=mx[:], scalar2=None, op0=OP.subtract
            )
            ex = rp.tile([P, OUT], F32, tag="ex")
            nc.scalar.activation(out=ex[:], in_=tm[:], func=AF.Exp)
            sm = sp.tile([P, 1], F32, tag="sm")
            nc.vector.reduce_sum(out=sm[:], in_=ex[:], axis=mybir.AxisListType.X)
            ls = sp.tile([P, 1], F32, tag="ls")
            nc.scalar.activation(out=ls[:], in_=sm[:], func=AF.Ln)
            fin = rp.tile([P, OUT], F32, tag="fin")
            nc.vector.tensor_scalar(
                out=fin[:], in0=tm[:], scalar1=ls[:], scalar2=None, op0=OP.subtract
            )
            nc.sync.dma_start(out2.ap()[t * P : (t + 1) * P, :], fin[:])

            oad += (ea + ept) // 16
            ob += eb // 16
            co += nchk
    nc.compile()
    return nc


def _prepare(x, edge_index, W1, a1_src, a1_dst, W2, a2_src, a2_dst):
    key = hash(np.asarray(edge_index).tobytes())
    if key in _CACHE:
        return _CACHE[key]
    EA, EB, streams = _prep_edges(edge_index)
    l1 = _build_l1(EA, EB)
    l2 = _build_l2(EA, EB)
    _CACHE.clear()
    _CACHE[key] = (EA, EB, streams, l1, l2)
    return _CACHE[key]


def _host_consts(x, W1, a1_src, a1_dst, W2, a2_src, a2_dst):
    x = np.asarray(x, np.float32)
    W1 = np.asarray(W1, np.float32)
    W2 = np.asarray(W2, np.float32)
    a1_src = np.asarray(a1_src, np.float32)
    a1_dst = np.asarray(a1_dst, np.float32)
    a2_src = np.asarray(a2_src, np.float32).reshape(-1)
    a2_dst = np.asarray(a2_dst, np.float32).reshape(-1)

    xpad = np.zeros((NPAD, IN), np.float32)
    xpad[:N] = x
    W1r = W1.reshape(IN, HEADS, HID)
    wsd = np.concatenate(
        [np.einsum("khc,hc->kh", W1r, a1_src), np.einsum("khc,hc->kh", W1r, a1_dst)],
        axis=1,
    )  # [128, 16]
    wv2s = W2 @ a2_src  # [512]
    wv2d = W2 @ a2_dst
    w2c = np.zeros((P, 4 * 42), np.float32)
    for j in range(4):
        w2c[:, j * 42 : j * 42 + 40] = W2[j * P : (j + 1) * P, :]
        w2c[:, j * 42 + 40] = wv2s[j * P : (j + 1) * P]
        w2c[:, j * 42 + 41] = wv2d[j * P : (j + 1) * P]
    iot = np.tile(np.arange(P, dtype=np.float32), (P, 1)).astype(_bf16)
    idn = np.eye(P, dtype=np.float32)
    return xpad, wsd.astype(_bf16), w2c.astype(_bf16), iot, idn.astype(_bf16), W1.astype(_bf16)


def _run(inputs, trace=False):
    x = inputs["x"]
    edge_index = inputs["edge_index"]
    EA, EB, streams, l1, l2 = _prepare(
        x, edge_index, inputs["W1"], inputs["a1_src"], inputs["a1_dst"],
        inputs["W2"], inputs["a2_src"], inputs["a2_dst"],
    )
    xpad, wsd, w2c, iot, idn, W1bf = _host_consts(
        x, inputs["W1"], inputs["a1_src"], inputs["a1_dst"],
        inputs["W2"], inputs["a2_src"], inputs["a2_dst"],
    )

    in_maps = []
    for c in range(NCORE):
        xr = np.roll(xpad, -c * SHARD, axis=0).astype(_bf16)
        s = streams[c]
        in_maps.append(
            dict(
                xtA=xr[:SPLIT], xtB=xr[SPLIT:],
                idxA=s["idxA"], idxB=s["idxB"], idxD=s["idxD"],
                dloc=np.ascontiguousarray(s["dloc"]),
                w1=W1bf, wsd=wsd, w2c=w2c, iot=iot, idn=idn,
            )
        )
    def _launch(prog, maps):
        try:
            return run_bass_kernel_spmd(prog, maps, list(range(NCORE)), trace=trace)
        except Exception:
            import time as _time
            _time.sleep(5)
            return run_bass_kernel_spmd(prog, maps, list(range(NCORE)), trace=trace)

    r1 = _launch(l1, in_maps)
    h2tab = np.zeros((NPAD, 64), np.float32)
    for c in range(NCORE):
        h2tab[c * SHARD : (c + 1) * SHARD] = r1.results[c]["h2row"]
    h2tab[N:] = 0.0

    in_maps2 = []
    for c in range(NCORE):
        hr = np.roll(h2tab, -c * SHARD, axis=0)
        s = streams[c]
        in_maps2.append(
            dict(
                htA=np.ascontiguousarray(hr[:SPLIT]),
                htB=np.ascontiguousarray(hr[SPLIT:]),
                idxAD=s["idxAD"], idxB=s["idxB"],
                dloc=np.ascontiguousarray(s["dloc"]), iot=iot,
            )
        )
    r2 = _launch(l2, in_maps2)
    out = np.concatenate([r2.results[c]["out2"] for c in range(NCORE)], axis=0)[:N]
    ns = None
    if r1.exec_time_ns is not None and r2.exec_time_ns is not None:
        ns = r1.exec_time_ns + r2.exec_time_ns
    return np.ascontiguousarray(out, dtype=np.float32), ns


def kernel(**inputs) -> np.ndarray:
    out, _ = _run(inputs, trace=False)
    return out



# revision 11
# speedup vs baseline: 2.5769x; 2.5769x over previous
"""GAT (2-layer, PyG-style) on 8 Trainium2 NeuronCores.

Three-launch pipeline; host does only indexing/layout between launches.

Node -> (core, tile, partition) assignment is degree-sorted so that each
128-node tile has near-uniform in-degree; edge chunk k of a tile holds the
k-th incoming edge of every tile node at that node's own partition
("identity slots"), so the segment scatter needs no one-hot build - the
scatter matmul uses a constant identity and the softmax denominator is a
per-partition free-axis reduction. Source rows are fetched with a single
signed-index dma_gather per tile (table rebased at row 32768 so int16
offsets span all 50176 rows; a 16-zero sentinel defeats the trailing-
negative-index early stop, which is per DMA queue).

L0: h = x@W1 (c-major column order) and attention dots es/ed per node.
L1: gather h[src] rows (bf16, 1 KB); p = exp(leakyrelu(es+ed)) from
    direct-DMA streams, batched per tile; 1/z is folded into the edge
    weights (alpha = p/z computed per tile before the value pass), so the
    weighting is one 2x-mode DVE multiply per chunk (c-major layout keeps
    every operand packed-last-dim) and the tile tail is just relu + the
    W2 block-matmuls. Emits the layer-2 node row [h2 | e2_src | e2_dst].
L2: same structure, 1 head, fp8 value table, 40-dim values; log_softmax
    batched over all 49 tiles at the end.
"""

import numpy as np
import ml_dtypes
from contextlib import ExitStack

import concourse.bass as bass
import concourse.mybir as mybir
import concourse.tile as tile
from concourse import bacc
from concourse.bass_utils import run_bass_kernel_spmd

F32 = mybir.dt.float32
BF16 = mybir.dt.bfloat16
FP8 = mybir.dt.float8e4
I16 = mybir.dt.int16
AF = mybir.ActivationFunctionType
OP = mybir.AluOpType

N = 50000
E = 500000
IN = 128
HID = 64
HEADS = 8
OUT = 40
NEG = 0.2
NCORE = 8
P = 128
TILES = 49
SHARD = TILES * P          # 6272
NPAD = NCORE * SHARD       # 50176
BASE = 32768               # signed-idx table rebase row
PADV = -200.0              # es pad -> p ~ 5e-18

_bf16 = ml_dtypes.bfloat16
_fp8 = ml_dtypes.float8_e4m3fn

_CACHE = {}

GCAP = 3584


def _gather(nc, out3, in_ap, idx_sb, col0, nreal, elem):
    """Signed-idx row gather of nreal+16 stream entries (16-zero sentinel)
    into out3 (sized nchk+1 chunks)."""
    n = nreal + 16
    done = 0
    while done < nreal:
        take = min(GCAP, n - done)
        nc.gpsimd.dma_gather(
            out_ap=out3[:, done // P : (done + take + P - 1) // P, :],
            in_ap=in_ap,
            idxs_ap=idx_sb[:, col0 + done // 16 : col0 + (done + take) // 16],
            num_idxs=take,
            num_idxs_reg=take,
            elem_size=elem,
            transpose=False,
            single_packet=False,
        )
        done += take


def _wrap16(v):
    assert len(v) % 16 == 0
    w = v.reshape(-1, 16).T.astype(np.int16)
    return np.tile(w, (8, 1))


def _prep_edges(edge_index):
    """Degree-sorted identity-slot layout.

    Returns NCHK (chunks per tile, SPMD-shared), and per-core:
      idx    [128, sum(NCHK)*8 + TILES] int16 gather stream (sentinels)
      srcn   [slots] source node id per slot (NPAD = pad sentinel)
      nodes  [TILES, 128] node id owning each (tile, partition)
    plus the global rank permutation `order` ([NPAD] node ids by rank).
    """
    src = np.concatenate([np.asarray(edge_index[0]), np.arange(N)]).astype(np.int64)
    dst = np.concatenate([np.asarray(edge_index[1]), np.arange(N)]).astype(np.int64)

    deg = np.bincount(dst, minlength=NPAD)
    order = np.argsort(-deg, kind="stable")          # rank -> node
    rank = np.empty(NPAD, np.int64)
    rank[order] = np.arange(NPAD)

    NCHK = [max(1, int(deg[order[t * 1024 : (t + 1) * 1024]].max()))
            for t in range(TILES)]
    base_t = np.concatenate([[0], np.cumsum([c * P for c in NCHK])])
    nslot = int(base_t[-1])

    r = rank[dst]
    t_e = r // 1024
    c_e = (r // P) % NCORE
    p_e = r % P
    # k = occurrence index of each edge within its dst
    o = np.argsort(r, kind="stable")
    rs = r[o]
    first = np.r_[True, rs[1:] != rs[:-1]]
    idx_in_run = np.arange(len(rs)) - np.maximum.accumulate(
        np.where(first, np.arange(len(rs)), 0)
    )
    k_e = np.empty(len(rs), np.int64)
    k_e[o] = idx_in_run

    pos = base_t[t_e] + k_e * P + p_e

    cores = []
    for c in range(NCORE):
        m = c_e == c
        srcn = np.full(nslot, NPAD, np.int64)        # NPAD = pad sentinel
        srcn[pos[m]] = src[m]
        ioff = np.zeros(nslot, np.int64)
        ioff[pos[m]] = src[m] - BASE
        # per-tile streams + sentinel, wrapped
        cols = []
        for t in range(TILES):
            seg = np.concatenate(
                [ioff[base_t[t] : base_t[t + 1]], np.zeros(16, np.int64)]
            )
            cols.append(_wrap16(seg))
        nodes = order.reshape(TILES, NCORE, P)[:, c, :]
        cores.append(dict(idx=np.concatenate(cols, axis=1), srcn=srcn,
                          nodes=nodes))
    return NCHK, cores, order


def _slotmaj(arr_slots):
    w = arr_slots.shape[1] if arr_slots.ndim == 2 else 1
    a = arr_slots.reshape(-1, P, w).transpose(1, 0, 2).reshape(P, -1)
    return np.ascontiguousarray(a)


def _build_l0():
    nc = bacc.Bacc("TRN2", target_bir_lowering=False, debug=False, num_devices=NCORE)
    xT = nc.dram_tensor("xT", [P, SHARD], BF16, kind="ExternalInput")
    w1 = nc.dram_tensor("w1", [P, HEADS * HID], BF16, kind="ExternalInput")
    wsd = nc.dram_tensor("wsd", [P, 2 * HEADS], BF16, kind="ExternalInput")
    hsh = nc.dram_tensor("hsh", [SHARD, HEADS * HID], BF16, kind="ExternalOutput")
    esd = nc.dram_tensor("esd", [SHARD, 2 * HEADS], F32, kind="ExternalOutput")

    GRP = 13  # DMA out in 4 groups to overlap with compute
    with tile.TileContext(nc) as tc, ExitStack() as ctx:
        cp = ctx.enter_context(tc.tile_pool(name="const", bufs=1))
        pp = ctx.enter_context(tc.tile_pool(name="ph", bufs=2, space="PSUM"))
        pe_ = ctx.enter_context(tc.tile_pool(name="pe", bufs=2, space="PSUM"))

        xsb = cp.tile([P, SHARD], BF16)
        nc.sync.dma_start(xsb[:], xT.ap())
        w1sb = cp.tile([P, HEADS * HID], BF16)
        nc.sync.dma_start(w1sb[:], w1.ap())
        wsdsb = cp.tile([P, 2 * HEADS], BF16)
        nc.sync.dma_start(wsdsb[:], wsd.ap())
        esb = cp.tile([P, TILES, 2 * HEADS], F32)
        hbuf = cp.tile([P, TILES, HEADS * HID], BF16)

        for t in range(TILES):
            lhs = xsb[:, t * P : (t + 1) * P]
            hps = pp.tile([P, HEADS * HID], F32, tag="h")
            nc.tensor.matmul(hps[:], lhsT=lhs, rhs=w1sb[:], start=True, stop=True)
            if t % 2 == 0:
                nc.scalar.activation(out=hbuf[:, t, :], in_=hps[:], func=AF.Copy)
            else:
                nc.vector.tensor_copy(out=hbuf[:, t, :], in_=hps[:])
            eps = pe_.tile([P, 2 * HEADS], F32, tag="e")
            nc.tensor.matmul(eps[:], lhsT=lhs, rhs=wsdsb[:], start=True, stop=True)
            nc.vector.tensor_copy(out=esb[:, t, :], in_=eps[:])
            if (t + 1) % GRP == 0 or t == TILES - 1:
                lo = (t // GRP) * GRP
                nc.sync.dma_start(
                    hsh.ap().rearrange("(t p) w -> p t w", p=P)[:, lo : t + 1, :],
                    hbuf[:, lo : t + 1, :],
                )
        nc.sync.dma_start(esd.ap().rearrange("(t p) w -> p t w", p=P), esb[:])
    nc.compile()
    return nc


def _build_l1(NCHK):
    nch = sum(NCHK)
    icols = nch * 8 + TILES

    nc = bacc.Bacc("TRN2", target_bir_lowering=False, debug=False, num_devices=NCORE)
    ht = nc.dram_tensor("ht", [NPAD + 1, HEADS * HID], BF16, kind="ExternalInput")
    idx = nc.dram_tensor("idx", [P, icols], I16, kind="ExternalInput")
    ess = nc.dram_tensor("ess", [P, nch * HEADS], BF16, kind="ExternalInput")
    edt = nc.dram_tensor("edt", [P, TILES * HEADS], BF16, kind="ExternalInput")
    w2cb = nc.dram_tensor("w2cb", [P, 4 * 42], BF16, kind="ExternalInput")
    idn = nc.dram_tensor("idn", [P, P], BF16, kind="ExternalInput")
    h2r = nc.dram_tensor("h2r", [SHARD, 42], F32, kind="ExternalOutput")

    with tile.TileContext(nc) as tc, ExitStack() as ctx:
        cp = ctx.enter_context(tc.tile_pool(name="const", bufs=1))
        gp = ctx.enter_context(tc.tile_pool(name="gath", bufs=3))
        sp = ctx.enter_context(tc.tile_pool(name="small", bufs=10))
        vp = ctx.enter_context(tc.tile_pool(name="vals", bufs=2))
        rp = ctx.enter_context(tc.tile_pool(name="tail", bufs=3))
        po = ctx.enter_context(tc.tile_pool(name="po", bufs=2, space="PSUM"))
        p2 = ctx.enter_context(tc.tile_pool(name="p2", bufs=2, space="PSUM"))

        isb = cp.tile([P, icols], I16)
        nc.sync.dma_start(isb[:], idx.ap())
        essb = cp.tile([P, nch * HEADS], BF16)
        nc.sync.dma_start(essb[:], ess.ap())
        edsb = cp.tile([P, TILES * HEADS], BF16)
        nc.sync.dma_start(edsb[:], edt.ap())
        w2sb = cp.tile([P, 4, 42], BF16)
        nc.sync.dma_start(w2sb[:], w2cb.ap().rearrange("p (b o) -> p b o", o=42))
        idsb = cp.tile([P, P], BF16)
        nc.sync.dma_start(idsb[:], idn.ap())
        h2buf = cp.tile([P, TILES, 42], F32)

        ic = co = 0
        for t in range(TILES):
            nchk = NCHK[t]
            gad = gp.tile([P, nchk + 1, HEADS * HID], BF16, tag="g")
            _gather(nc, gad, ht.ap()[BASE:, :], isb, ic, nchk * P, HEADS * HID)

            w = nchk * HEADS
            c8 = co * HEADS
            # p pipeline (batched per tile) + alpha = p/z folding
            lg = sp.tile([P, nchk, HEADS], BF16, tag="lg")
            nc.vector.tensor_tensor(
                out=lg[:],
                in0=essb[:, c8 : c8 + w].rearrange("p (k h) -> p k h", h=HEADS),
                in1=edsb[:, t * HEADS : (t + 1) * HEADS]
                    .unsqueeze(1).to_broadcast([P, nchk, HEADS]),
                op=OP.add,
            )
            pl = sp.tile([P, w], BF16, tag="pl")
            nc.scalar.activation(
                out=pl[:], in_=lg[:].rearrange("p k h -> p (k h)"),
                func=AF.Prelu, alpha=NEG,
            )
            p32 = sp.tile([P, w], F32, tag="p32")
            nc.scalar.activation(out=p32[:], in_=pl[:], func=AF.Exp)
            z = sp.tile([P, HEADS], F32, tag="z")
            nc.vector.tensor_reduce(
                out=z[:],
                in_=p32[:].rearrange("p (k h) -> p h k", h=HEADS),
                op=OP.add, axis=mybir.AxisListType.X,
            )
            zr = sp.tile([P, HEADS], F32, tag="zr")
            nc.vector.reciprocal(zr[:], z[:])
            abf = sp.tile([P, nchk, HEADS], BF16, tag="abf")
            nc.vector.tensor_tensor(
                out=abf[:],
                in0=p32[:].rearrange("p (k h) -> p k h", h=HEADS),
                in1=zr[:].unsqueeze(1).to_broadcast([P, nchk, HEADS]),
                op=OP.mult,
            )

            # batched value weighting (2x-mode: every operand packed-last)
            vt = vp.tile([P, nchk, HID, HEADS], BF16, tag="vt")
            nc.vector.tensor_tensor(
                out=vt[:],
                in0=gad[:, 0:nchk, :].rearrange("p k (c h) -> p k c h", h=HEADS),
                in1=abf[:].unsqueeze(2).to_broadcast([P, nchk, HID, HEADS]),
                op=OP.mult,
            )
            vtf = vt[:].rearrange("p k c h -> p k (c h)")
            # one PSUM accumulation group per bank (start zeroes whole bank)
            r1T = rp.tile([P, 4, P], BF16, tag="r1T")
            for b in range(4):
                o1T = po.tile([P, 512], F32, tag="o1T")
                for k in range(nchk):
                    nc.tensor.matmul(
                        o1T[:, 0:P], lhsT=vtf[:, k, b * P : (b + 1) * P],
                        rhs=idsb[:], start=(k == 0), stop=(k == nchk - 1),
                    )
                nc.scalar.activation(out=r1T[:, b, :], in_=o1T[:, 0:P],
                                     func=AF.Relu)
            h2ps = p2.tile([P, 42], F32, tag="h2")
            for b in range(4):
                nc.tensor.matmul(
                    h2ps[:], lhsT=r1T[:, b, :], rhs=w2sb[:, b, :],
                    start=(b == 0), stop=(b == 3),
                )
            nc.vector.tensor_copy(out=h2buf[:, t, :], in_=h2ps[:])
            ic += nchk * 8 + 1
            co += nchk

        nc.sync.dma_start(h2r.ap().rearrange("(t p) o -> p t o", p=P), h2buf[:])
    nc.compile()
    return nc


def _build_l2(NCHK):
    nch = sum(NCHK)
    icols = nch * 8 + TILES

    nc = bacc.Bacc("TRN2", target_bir_lowering=False, debug=False, num_devices=NCORE)
    ht2 = nc.dram_tensor("ht2", [NPAD + 1, P], BF16, kind="ExternalInput")
    idx2 = nc.dram_tensor("idx2", [P, icols], I16, kind="ExternalInput")
    e2s = nc.dram_tensor("e2s", [P, nch], BF16, kind="ExternalInput")
    e2dt = nc.dram_tensor("e2dt", [P, TILES], F32, kind="ExternalInput")
    idn2 = nc.dram_tensor("idn2", [P, P], BF16, kind="ExternalInput")
    out2 = nc.dram_tensor("out2", [SHARD, OUT], F32, kind="ExternalOutput")

    with tile.TileContext(nc) as tc, ExitStack() as ctx:
        cp = ctx.enter_context(tc.tile_pool(name="const", bufs=1))
        gp = ctx.enter_context(tc.tile_pool(name="gath", bufs=3))
        sp = ctx.enter_context(tc.tile_pool(name="small", bufs=10))
        vp = ctx.enter_context(tc.tile_pool(name="vals", bufs=6))
        po = ctx.enter_context(tc.tile_pool(name="po", bufs=2, space="PSUM"))

        isb = cp.tile([P, icols], I16)
        nc.sync.dma_start(isb[:], idx2.ap())
        essb = cp.tile([P, nch], BF16)
        nc.sync.dma_start(essb[:], e2s.ap())
        edsb = cp.tile([P, TILES], F32)
        nc.sync.dma_start(edsb[:], e2dt.ap())
        idsb = cp.tile([P, P], BF16)
        nc.sync.dma_start(idsb[:], idn2.ap())
        avbuf = cp.tile([P, TILES, OUT], F32)

        ic = co = 0
        for t in range(TILES):
            nchk = NCHK[t]
            gad = gp.tile([P, nchk + 1, P], BF16, tag="g")
            _gather(nc, gad, ht2.ap()[BASE:, :], isb, ic, nchk * P, P)

            lg = sp.tile([P, nchk], BF16, tag="lg")
            nc.vector.tensor_scalar(
                out=lg[:], in0=essb[:, co : co + nchk],
                scalar1=edsb[:, t : t + 1], scalar2=None, op0=OP.add,
            )
            pl = sp.tile([P, nchk], BF16, tag="pl")
            nc.scalar.activation(out=pl[:], in_=lg[:], func=AF.Prelu, alpha=NEG)
            p32 = sp.tile([P, nchk], F32, tag="p32")
            nc.scalar.activation(out=p32[:], in_=pl[:], func=AF.Exp)
            z = sp.tile([P, 1], F32, tag="z")
            nc.vector.tensor_reduce(
                out=z[:], in_=p32[:], op=OP.add, axis=mybir.AxisListType.X,
            )
            zr = sp.tile([P, 1], F32, tag="zr")
            nc.vector.reciprocal(zr[:], z[:])
            a32 = sp.tile([P, nchk], F32, tag="a32")
            nc.vector.tensor_scalar(
                out=a32[:], in0=p32[:], scalar1=zr[:], scalar2=None, op0=OP.mult,
            )

            o2ps = po.tile([P, OUT], F32, tag="o2")
            for k in range(nchk):
                v2 = vp.tile([P, OUT], BF16, tag="v2")
                nc.vector.tensor_scalar(
                    out=v2[:], in0=gad[:, k, 0:OUT],
                    scalar1=a32[:, k : k + 1], scalar2=None, op0=OP.mult,
                )
                nc.tensor.matmul(
                    o2ps[:], lhsT=idsb[:], rhs=v2[:],
                    start=(k == 0), stop=(k == nchk - 1),
                )
            nc.vector.tensor_copy(out=avbuf[:, t, :], in_=o2ps[:])
            ic += nchk * 8 + 1
            co += nchk

        # batched log_softmax over all tiles
        mx = cp.tile([P, TILES], F32)
        nc.vector.reduce_max(out=mx[:], in_=avbuf[:], axis=mybir.AxisListType.X)
        tm = cp.tile([P, TILES, OUT], F32)
        nc.vector.tensor_tensor(
            out=tm[:], in0=avbuf[:],
            in1=mx[:].unsqueeze(2).to_broadcast([P, TILES, OUT]), op=OP.subtract,
        )
        ex = cp.tile([P, TILES, OUT], F32)
        nc.scalar.activation(out=ex[:], in_=tm[:], func=AF.Exp)
        sm = cp.tile([P, TILES], F32)
        nc.vector.reduce_sum(out=sm[:], in_=ex[:], axis=mybir.AxisListType.X)
        ls = cp.tile([P, TILES], F32)
        nc.scalar.activation(out=ls[:], in_=sm[:], func=AF.Ln)
        fin = cp.tile([P, TILES, OUT], F32)
        nc.vector.tensor_tensor(
            out=fin[:], in0=tm[:],
            in1=ls[:].unsqueeze(2).to_broadcast([P, TILES, OUT]), op=OP.subtract,
        )
        nc.sync.dma_start(out2.ap().rearrange("(t p) o -> p t o", p=P), fin[:])
    nc.compile()
    return nc


def _prepare(edge_index):
    key = hash(np.asarray(edge_index).tobytes())
    if key in _CACHE:
        return _CACHE[key]
    NCHK, cores, order = _prep_edges(edge_index)
    l0 = _build_l0()
    l1 = _build_l1(NCHK)
    l2 = _build_l2(NCHK)
    _CACHE.clear()
    _CACHE[key] = (NCHK, cores, order, l0, l1, l2)
    return _CACHE[key]


# c-major permutation: cm column j = c*8+h  <->  original column h*64+c
_CMJ = np.arange(HEADS * HID)
_CM_FROM = (_CMJ % HEADS) * HID + _CMJ // HEADS   # cm[:, j] = orig[:, _CM_FROM[j]]


def _host_consts(W1, a1_src, a1_dst, W2, a2_src, a2_dst):
    W1 = np.asarray(W1, np.float32)
    W2 = np.asarray(W2, np.float32)
    a1_src = np.asarray(a1_src, np.float32)
    a1_dst = np.asarray(a1_dst, np.float32)
    a2_src = np.asarray(a2_src, np.float32).reshape(-1)
    a2_dst = np.asarray(a2_dst, np.float32).reshape(-1)

    W1r = W1.reshape(IN, HEADS, HID)
    wsd = np.concatenate(
        [np.einsum("khc,hc->kh", W1r, a1_src), np.einsum("khc,hc->kh", W1r, a1_dst)],
        axis=1,
    )
    W1cm = W1[:, _CM_FROM]
    wv2s = W2 @ a2_src
    wv2d = W2 @ a2_dst
    W2C = np.concatenate([W2, wv2s[:, None], wv2d[:, None]], axis=1)  # [512,42]
    W2Ccm = W2C[_CM_FROM, :]
    w2cb = np.ascontiguousarray(
        W2Ccm.reshape(4, P, 42).transpose(1, 0, 2).reshape(P, 4 * 42)
    )
    return W1cm, wsd, w2cb


def _launch(prog, maps, trace=False):
    try:
        return run_bass_kernel_spmd(prog, maps, list(range(NCORE)), trace=trace)
    except Exception:
        import time as _time
        _time.sleep(5)
        return run_bass_kernel_spmd(prog, maps, list(range(NCORE)), trace=trace)


def _run(inputs, trace=False):
    x = np.asarray(inputs["x"], np.float32)
    edge_index = inputs["edge_index"]
    NCHK, cores, order, l0, l1, l2 = _prepare(edge_index)
    W1cm, wsd, w2cb = _host_consts(
        inputs["W1"], inputs["a1_src"], inputs["a1_dst"],
        inputs["W2"], inputs["a2_src"], inputs["a2_dst"],
    )
    W1bf = W1cm.astype(_bf16)
    wsdbf = wsd.astype(_bf16)
    w2cbf = w2cb.astype(_bf16)
    idn = np.eye(P, dtype=np.float32).astype(_bf16)

    xpad = np.zeros((NPAD, IN), np.float32)
    xpad[:N] = x

    # ---- L0 ----------------------------------------------------------------
    maps0 = []
    for c in range(NCORE):
        xT = np.ascontiguousarray(xpad[c * SHARD : (c + 1) * SHARD].T.astype(_bf16))
        maps0.append(dict(xT=xT, w1=W1bf, wsd=wsdbf))
    r0 = _launch(l0, maps0, trace)
    hfull = np.concatenate([r0.results[c]["hsh"] for c in range(NCORE)], axis=0)
    esd = np.concatenate([r0.results[c]["esd"] for c in range(NCORE)], axis=0)

    ht = np.zeros((NPAD + 1, HEADS * HID), _bf16)
    ht[:NPAD] = hfull  # already c-major (W1cm) and bf16
    es_ext = np.full((NPAD + 1, HEADS), PADV, np.float32)
    es_ext[:NPAD] = esd[:, :HEADS]
    ed_full = esd[:, HEADS:]

    # ---- L1 ----------------------------------------------------------------
    maps1 = []
    for c in range(NCORE):
        s = cores[c]
        ess = _slotmaj(es_ext[s["srcn"]]).astype(_bf16)
        edtile = np.ascontiguousarray(
            ed_full[s["nodes"]].transpose(1, 0, 2).reshape(P, TILES * HEADS)
        ).astype(_bf16)
        maps1.append(dict(ht=ht, idx=s["idx"], ess=ess, edt=edtile,
                          w2cb=w2cbf, idn=idn))
    r1 = _launch(l1, maps1, trace)

    h2full = np.zeros((NPAD, 42), np.float32)
    for c in range(NCORE):
        nodes = cores[c]["nodes"].reshape(-1)          # slot (t*128+p) -> node
        h2full[nodes] = r1.results[c]["h2r"]
    h2full[N:] = 0.0

    # ---- L2 ----------------------------------------------------------------
    ht2 = np.zeros((NPAD + 1, P), _bf16)
    ht2[:NPAD, :OUT] = h2full[:, :OUT].astype(_bf16)
    e2s_ext = np.full(NPAD + 1, PADV, np.float32)
    e2s_ext[:NPAD] = h2full[:, 40]
    e2d_n = h2full[:, 41]
    maps2 = []
    for c in range(NCORE):
        s = cores[c]
        e2sv = _slotmaj(e2s_ext[s["srcn"]][:, None]).astype(_bf16)
        e2dt = np.ascontiguousarray(
            e2d_n[s["nodes"]].T
        ).astype(np.float32)
        maps2.append(dict(ht2=ht2, idx2=s["idx"], e2s=e2sv, e2dt=e2dt, idn2=idn))
    r2 = _launch(l2, maps2, trace)

    out = np.zeros((NPAD, OUT), np.float32)
    for c in range(NCORE):
        nodes = cores[c]["nodes"].reshape(-1)
        out[nodes] = r2.results[c]["out2"]
    out = out[:N]
    ns = None
    if all(r.exec_time_ns is not None for r in (r0, r1, r2)):
        ns = r0.exec_time_ns + r1.exec_time_ns + r2.exec_time_ns
    return np.ascontiguousarray(out, dtype=np.float32), ns


def kernel(**inputs) -> np.ndarray:
    out, _ = _run(inputs, trace=False)
    return out


# revision 20
# speedup vs baseline: 2.7283x; 1.0588x over previous
"""GAT (2-layer, PyG-style) on 8 Trainium2 NeuronCores.

Three-launch pipeline; host does only indexing/layout between launches.

Node -> (core, tile, partition) assignment is degree-sorted so that each
128-node tile has near-uniform in-degree; edge chunk k of a tile holds the
k-th incoming edge of every tile node at that node's own partition
("identity slots"), so the segment scatter needs no one-hot build - the
scatter matmul uses a constant identity and the softmax denominator is a
per-partition free-axis reduction. Source rows are fetched with a single
signed-index dma_gather per tile (table rebased at row 32768 so int16
offsets span all 50176 rows; a 16-zero sentinel defeats the trailing-
negative-index early stop, which is per DMA queue).

L0: h = x@W1 (c-major column order) and attention dots es/ed per node.
L1: gather h[src] rows (bf16, 1 KB); p = exp(leakyrelu(es+ed)) from
    direct-DMA streams, batched per tile; 1/z is folded into the edge
    weights (alpha = p/z computed per tile before the value pass), so the
    weighting is one 2x-mode DVE multiply per chunk (c-major layout keeps
    every operand packed-last-dim) and the tile tail is just relu + the
    W2 block-matmuls. Emits the layer-2 node row [h2 | e2_src | e2_dst].
L2: same structure, 1 head, fp8 value table, 40-dim values; log_softmax
    batched over all 49 tiles at the end.
"""

import numpy as np
import ml_dtypes
from contextlib import ExitStack

import concourse.bass as bass
import concourse.mybir as mybir
import concourse.tile as tile
from concourse import bacc
from concourse.bass_utils import run_bass_kernel_spmd

F32 = mybir.dt.float32
BF16 = mybir.dt.bfloat16
FP8 = mybir.dt.float8e4
I16 = mybir.dt.int16
AF = mybir.ActivationFunctionType
OP = mybir.AluOpType

N = 50000
E = 500000
IN = 128
HID = 64
HEADS = 8
OUT = 40
NEG = 0.2
NCORE = 8
P = 128
TILES = 49
SHARD = TILES * P          # 6272
NPAD = NCORE * SHARD       # 50176
BASE = 32768               # signed-idx table rebase row
PADV = -200.0              # es pad -> p ~ 5e-18

_bf16 = ml_dtypes.bfloat16
_fp8 = ml_dtypes.float8_e4m3fn

_CACHE = {}

GCAP = 3584


def _gather(nc, out3, in_ap, idx_sb, col0, nreal, elem):
    """Signed-idx row gather of nreal+16 stream entries (16-zero sentinel)
    into out3 (sized nchk+1 chunks)."""
    n = nreal + 16
    done = 0
    while done < nreal:
        take = min(GCAP, n - done)
        nc.gpsimd.dma_gather(
            out_ap=out3[:, done // P : (done + take + P - 1) // P, :],
            in_ap=in_ap,
            idxs_ap=idx_sb[:, col0 + done // 16 : col0 + (done + take) // 16],
            num_idxs=take,
            num_idxs_reg=take,
            elem_size=elem,
            transpose=False,
            single_packet=False,
        )
        done += take


def _wrap16(v):
    assert len(v) % 16 == 0
    w = v.reshape(-1, 16).T.astype(np.int16)
    return np.tile(w, (8, 1))


def _prep_edges(edge_index):
    """Degree-sorted identity-slot layout.

    Returns NCHK (chunks per tile, SPMD-shared), and per-core:
      idx    [128, sum(NCHK)*8 + TILES] int16 gather stream (sentinels)
      srcn   [slots] source node id per slot (NPAD = pad sentinel)
      nodes  [TILES, 128] node id owning each (tile, partition)
    plus the global rank permutation `order` ([NPAD] node ids by rank).
    """
    src = np.concatenate([np.asarray(edge_index[0]), np.arange(N)]).astype(np.int64)
    dst = np.concatenate([np.asarray(edge_index[1]), np.arange(N)]).astype(np.int64)

    deg = np.bincount(dst, minlength=NPAD)
    order = np.argsort(-deg, kind="stable")          # rank -> node
    rank = np.empty(NPAD, np.int64)
    rank[order] = np.arange(NPAD)

    NCHK = [max(1, int(deg[order[t * 1024 : (t + 1) * 1024]].max()))
            for t in range(TILES)]
    base_t = np.concatenate([[0], np.cumsum([c * P for c in NCHK])])
    nslot = int(base_t[-1])

    r = rank[dst]
    t_e = r // 1024
    c_e = (r // P) % NCORE
    p_e = r % P
    # k = occurrence index of each edge within its dst
    o = np.argsort(r, kind="stable")
    rs = r[o]
    first = np.r_[True, rs[1:] != rs[:-1]]
    idx_in_run = np.arange(len(rs)) - np.maximum.accumulate(
        np.where(first, np.arange(len(rs)), 0)
    )
    k_e = np.empty(len(rs), np.int64)
    k_e[o] = idx_in_run

    pos = base_t[t_e] + k_e * P + p_e

    cores = []
    for c in range(NCORE):
        m = c_e == c
        srcn = np.full(nslot, NPAD, np.int64)        # NPAD = pad sentinel
        srcn[pos[m]] = src[m]
        ioff = np.zeros(nslot, np.int64)
        ioff[pos[m]] = src[m] - BASE
        # per-tile streams + sentinel, wrapped
        cols = []
        for t in range(TILES):
            seg = np.concatenate(
                [ioff[base_t[t] : base_t[t + 1]], np.zeros(16, np.int64)]
            )
            cols.append(_wrap16(seg))
        nodes = order.reshape(TILES, NCORE, P)[:, c, :]
        cores.append(dict(idx=np.concatenate(cols, axis=1), srcn=srcn,
                          nodes=nodes))
    return NCHK, cores, order


def _slotmaj(arr_slots):
    w = arr_slots.shape[1] if arr_slots.ndim == 2 else 1
    a = arr_slots.reshape(-1, P, w).transpose(1, 0, 2).reshape(P, -1)
    return np.ascontiguousarray(a)


def _build_l0():
    nc = bacc.Bacc("TRN2", target_bir_lowering=False, debug=False, num_devices=NCORE)
    xT = nc.dram_tensor("xT", [P, SHARD], BF16, kind="ExternalInput")
    w1 = nc.dram_tensor("w1", [P, HEADS * HID], BF16, kind="ExternalInput")
    wsd = nc.dram_tensor("wsd", [P, 2 * HEADS], BF16, kind="ExternalInput")
    hsh = nc.dram_tensor("hsh", [SHARD, HEADS * HID], BF16, kind="ExternalOutput")
    esd = nc.dram_tensor("esd", [SHARD, 2 * HEADS], F32, kind="ExternalOutput")

    GRP = 7  # DMA out in groups to overlap with compute
    with tile.TileContext(nc) as tc, ExitStack() as ctx:
        cp = ctx.enter_context(tc.tile_pool(name="const", bufs=1))
        pp = ctx.enter_context(tc.tile_pool(name="ph", bufs=3, space="PSUM"))
        pe_ = ctx.enter_context(tc.tile_pool(name="pe", bufs=3, space="PSUM"))

        xsb = cp.tile([P, SHARD], BF16)
        for q in range(4):
            qs = (SHARD // 4) * q
            qe = SHARD if q == 3 else (SHARD // 4) * (q + 1)
            nc.sync.dma_start(xsb[:, qs:qe], xT.ap()[:, qs:qe])
        w1sb = cp.tile([P, HEADS * HID], BF16)
        nc.sync.dma_start(w1sb[:], w1.ap())
        wsdsb = cp.tile([P, 2 * HEADS], BF16)
        nc.sync.dma_start(wsdsb[:], wsd.ap())
        esb = cp.tile([P, TILES, 2 * HEADS], F32)
        hbuf = cp.tile([P, TILES, HEADS * HID], BF16)

        for t in range(TILES):
            lhs = xsb[:, t * P : (t + 1) * P]
            hps = pp.tile([P, HEADS * HID], F32, tag="h")
            nc.tensor.matmul(hps[:], lhsT=lhs, rhs=w1sb[:], start=True, stop=True)
            if t % 2 == 0:
                nc.scalar.activation(out=hbuf[:, t, :], in_=hps[:], func=AF.Copy)
            else:
                nc.vector.tensor_copy(out=hbuf[:, t, :], in_=hps[:])
            eps = pe_.tile([P, 2 * HEADS], F32, tag="e")
            nc.tensor.matmul(eps[:], lhsT=lhs, rhs=wsdsb[:], start=True, stop=True)
            if t % 2 == 0:
                nc.vector.tensor_copy(out=esb[:, t, :], in_=eps[:])
            else:
                nc.scalar.activation(out=esb[:, t, :], in_=eps[:], func=AF.Copy)
            if (t + 1) % GRP == 0 or t == TILES - 1:
                lo = (t // GRP) * GRP
                nc.sync.dma_start(
                    hsh.ap().rearrange("(t p) w -> p t w", p=P)[:, lo : t + 1, :],
                    hbuf[:, lo : t + 1, :],
                )
        nc.sync.dma_start(esd.ap().rearrange("(t p) w -> p t w", p=P), esb[:])
    nc.compile()
    return nc


def _build_l1(NCHK):
    nch = sum(NCHK)
    icols = nch * 8 + TILES

    nc = bacc.Bacc("TRN2", target_bir_lowering=False, debug=False, num_devices=NCORE)
    ht = nc.dram_tensor("ht", [NPAD + 1, HEADS * HID], BF16, kind="ExternalInput")
    idx = nc.dram_tensor("idx", [P, icols], I16, kind="ExternalInput")
    ess = nc.dram_tensor("ess", [P, nch * HEADS], BF16, kind="ExternalInput")
    edt = nc.dram_tensor("edt", [P, TILES * HEADS], BF16, kind="ExternalInput")
    w2cb = nc.dram_tensor("w2cb", [P, 4 * 42], BF16, kind="ExternalInput")
    idn = nc.dram_tensor("idn", [P, P], BF16, kind="ExternalInput")
    h2r = nc.dram_tensor("h2r", [SHARD, 42], F32, kind="ExternalOutput")

    with tile.TileContext(nc) as tc, ExitStack() as ctx:
        cp = ctx.enter_context(tc.tile_pool(name="const", bufs=1))
        gp = ctx.enter_context(tc.tile_pool(name="gath", bufs=3))
        sp = ctx.enter_context(tc.tile_pool(name="small", bufs=10))
        vp = ctx.enter_context(tc.tile_pool(name="vals", bufs=2))
        rp = ctx.enter_context(tc.tile_pool(name="tail", bufs=3))
        po = ctx.enter_context(tc.tile_pool(name="po", bufs=2, space="PSUM"))
        p2 = ctx.enter_context(tc.tile_pool(name="p2", bufs=2, space="PSUM"))

        c3 = sum(NCHK[:3]) * 8 + 3
        w3 = sum(NCHK[:3]) * HEADS
        isb = cp.tile([P, icols], I16)
        nc.sync.dma_start(isb[:, 0:c3], idx.ap()[:, 0:c3])
        essb = cp.tile([P, nch * HEADS], BF16)
        nc.sync.dma_start(essb[:, 0:w3], ess.ap()[:, 0:w3])
        edsb = cp.tile([P, TILES * HEADS], BF16)
        nc.sync.dma_start(edsb[:], edt.ap())
        w2sb = cp.tile([P, 4, 42], BF16)
        nc.sync.dma_start(w2sb[:], w2cb.ap().rearrange("p (b o) -> p b o", o=42))
        idsb = cp.tile([P, P], BF16)
        nc.sync.dma_start(idsb[:], idn.ap())
        h2buf = cp.tile([P, TILES, 42], F32)

        ic = co = 0
        for t in range(TILES):
            nchk = NCHK[t]
            if t == 2:
                nc.sync.dma_start(isb[:, c3:], idx.ap()[:, c3:])
                nc.sync.dma_start(essb[:, w3:], ess.ap()[:, w3:])
            gad = gp.tile([P, nchk + 1, HEADS * HID], BF16, tag="g")
            _gather(nc, gad, ht.ap()[BASE:, :], isb, ic, nchk * P, HEADS * HID)

            w = nchk * HEADS
            c8 = co * HEADS
            # p pipeline (batched per tile) + alpha = p/z folding
            lg = sp.tile([P, nchk, HEADS], BF16, tag="lg")
            nc.vector.tensor_tensor(
                out=lg[:],
                in0=essb[:, c8 : c8 + w].rearrange("p (k h) -> p k h", h=HEADS),
                in1=edsb[:, t * HEADS : (t + 1) * HEADS]
                    .unsqueeze(1).to_broadcast([P, nchk, HEADS]),
                op=OP.add,
            )
            pl = sp.tile([P, w], BF16, tag="pl")
            nc.scalar.activation(
                out=pl[:], in_=lg[:].rearrange("p k h -> p (k h)"),
                func=AF.Prelu, alpha=NEG,
            )
            p32 = sp.tile([P, w], F32, tag="p32")
            nc.scalar.activation(out=p32[:], in_=pl[:], func=AF.Exp)
            z = sp.tile([P, HEADS], F32, tag="z")
            nc.vector.tensor_reduce(
                out=z[:],
                in_=p32[:].rearrange("p (k h) -> p h k", h=HEADS),
                op=OP.add, axis=mybir.AxisListType.X,
            )
            zr = sp.tile([P, HEADS], F32, tag="zr")
            nc.vector.reciprocal(zr[:], z[:])
            abf = sp.tile([P, nchk, HEADS], BF16, tag="abf")
            nc.vector.tensor_tensor(
                out=abf[:],
                in0=p32[:].rearrange("p (k h) -> p k h", h=HEADS),
                in1=zr[:].unsqueeze(1).to_broadcast([P, nchk, HEADS]),
                op=OP.mult,
            )

            # batched value weighting (2x-mode: every operand packed-last)
            vt = vp.tile([P, nchk, HID, HEADS], BF16, tag="vt")
            nc.vector.tensor_tensor(
                out=vt[:],
                in0=gad[:, 0:nchk, :].rearrange("p k (c h) -> p k c h", h=HEADS),
                in1=abf[:].unsqueeze(2).to_broadcast([P, nchk, HID, HEADS]),
                op=OP.mult,
            )
            vtf = vt[:].rearrange("p k c h -> p k (c h)")
            # one PSUM accumulation group per bank (start zeroes whole bank)
            r1T = rp.tile([P, 4, P], BF16, tag="r1T")
            for b in range(4):
                o1T = po.tile([P, 512], F32, tag="o1T")
                for k in range(nchk):
                    nc.tensor.matmul(
                        o1T[:, 0:P], lhsT=vtf[:, k, b * P : (b + 1) * P],
                        rhs=idsb[:], start=(k == 0), stop=(k == nchk - 1),
                    )
                nc.scalar.activation(out=r1T[:, b, :], in_=o1T[:, 0:P],
                                     func=AF.Relu)
            h2ps = p2.tile([P, 42], F32, tag="h2")
            for b in range(4):
                nc.tensor.matmul(
                    h2ps[:], lhsT=r1T[:, b, :], rhs=w2sb[:, b, :],
                    start=(b == 0), stop=(b == 3),
                )
            nc.scalar.activation(out=h2buf[:, t, :], in_=h2ps[:], func=AF.Copy)
            if (t + 1) % 12 == 0 or t == TILES - 1:
                lo = (t // 12) * 12
                nc.sync.dma_start(
                    h2r.ap().rearrange("(t p) o -> p t o", p=P)[:, lo : t + 1, :],
                    h2buf[:, lo : t + 1, :],
                )
            ic += nchk * 8 + 1
            co += nchk

    nc.compile()
    return nc


def _build_l2(NCHK):
    nch = sum(NCHK)
    icols = nch * 8 + TILES

    nc = bacc.Bacc("TRN2", target_bir_lowering=False, debug=False, num_devices=NCORE)
    ht2 = nc.dram_tensor("ht2", [NPAD + 1, P], BF16, kind="ExternalInput")
    idx2 = nc.dram_tensor("idx2", [P, icols], I16, kind="ExternalInput")
    e2s = nc.dram_tensor("e2s", [P, nch], BF16, kind="ExternalInput")
    e2dt = nc.dram_tensor("e2dt", [P, TILES], F32, kind="ExternalInput")
    idn2 = nc.dram_tensor("idn2", [P, P], BF16, kind="ExternalInput")
    out2 = nc.dram_tensor("out2", [SHARD, OUT], F32, kind="ExternalOutput")

    with tile.TileContext(nc) as tc, ExitStack() as ctx:
        cp = ctx.enter_context(tc.tile_pool(name="const", bufs=1))
        gp = ctx.enter_context(tc.tile_pool(name="gath", bufs=4))
        sp = ctx.enter_context(tc.tile_pool(name="small", bufs=10))
        vp = ctx.enter_context(tc.tile_pool(name="vals", bufs=6))
        po = ctx.enter_context(tc.tile_pool(name="po", bufs=2, space="PSUM"))

        c3 = sum(NCHK[:3]) * 8 + 3
        w3 = sum(NCHK[:3])
        isb = cp.tile([P, icols], I16)
        nc.sync.dma_start(isb[:, 0:c3], idx2.ap()[:, 0:c3])
        essb = cp.tile([P, nch], BF16)
        nc.sync.dma_start(essb[:, 0:w3], e2s.ap()[:, 0:w3])
        edsb = cp.tile([P, TILES], F32)
        nc.sync.dma_start(edsb[:], e2dt.ap())
        idsb = cp.tile([P, P], BF16)
        nc.sync.dma_start(idsb[:], idn2.ap())
        avbuf = cp.tile([P, TILES, OUT], F32)
        smbuf = cp.tile([P, TILES], F32)

        ic = co = 0
        for t in range(TILES):
            nchk = NCHK[t]
            if t == 2:
                nc.sync.dma_start(isb[:, c3:], idx2.ap()[:, c3:])
                nc.sync.dma_start(essb[:, w3:], e2s.ap()[:, w3:])
            gad = gp.tile([P, nchk + 1, P], BF16, tag="g")
            _gather(nc, gad, ht2.ap()[BASE:, :], isb, ic, nchk * P, P)

            lg = sp.tile([P, nchk], BF16, tag="lg")
            nc.vector.tensor_scalar(
                out=lg[:], in0=essb[:, co : co + nchk],
                scalar1=edsb[:, t : t + 1], scalar2=None, op0=OP.add,
            )
            pl = sp.tile([P, nchk], BF16, tag="pl")
            nc.scalar.activation(out=pl[:], in_=lg[:], func=AF.Prelu, alpha=NEG)
            p32 = sp.tile([P, nchk], F32, tag="p32")
            nc.scalar.activation(out=p32[:], in_=pl[:], func=AF.Exp)
            z = sp.tile([P, 1], F32, tag="z")
            nc.vector.tensor_reduce(
                out=z[:], in_=p32[:], op=OP.add, axis=mybir.AxisListType.X,
            )
            zr = sp.tile([P, 1], F32, tag="zr")
            nc.vector.reciprocal(zr[:], z[:])
            a32 = sp.tile([P, nchk], F32, tag="a32")
            nc.vector.tensor_scalar(
                out=a32[:], in0=p32[:], scalar1=zr[:], scalar2=None, op0=OP.mult,
            )

            o2ps = po.tile([P, OUT], F32, tag="o2")
            for k in range(nchk):
                v2 = vp.tile([P, OUT], BF16, tag="v2")
                nc.vector.tensor_scalar(
                    out=v2[:], in0=gad[:, k, 0:OUT],
                    scalar1=a32[:, k : k + 1], scalar2=None, op0=OP.mult,
                )
                nc.tensor.matmul(
                    o2ps[:], lhsT=idsb[:], rhs=v2[:],
                    start=(k == 0), stop=(k == nchk - 1),
                )
            # per-tile log_softmax prefix (Exp only; Ln batched at end to
            # avoid act-table thrash)
            mx = sp.tile([P, 1], F32, tag="mx")
            nc.vector.reduce_max(out=mx[:], in_=o2ps[:], axis=mybir.AxisListType.X)
            nc.vector.tensor_scalar(
                out=avbuf[:, t, :], in0=o2ps[:], scalar1=mx[:], scalar2=None,
                op0=OP.subtract,
            )
            ex = sp.tile([P, OUT], F32, tag="ex")
            nc.scalar.activation(out=ex[:], in_=avbuf[:, t, :], func=AF.Exp)
            nc.vector.reduce_sum(out=smbuf[:, t : t + 1], in_=ex[:],
                                 axis=mybir.AxisListType.X)
            ic += nchk * 8 + 1
            co += nchk

        ls = cp.tile([P, TILES], F32)
        nc.scalar.activation(out=ls[:], in_=smbuf[:], func=AF.Ln)
        fin = cp.tile([P, TILES, OUT], F32)
        nc.vector.tensor_tensor(
            out=fin[:], in0=avbuf[:],
            in1=ls[:].unsqueeze(2).to_broadcast([P, TILES, OUT]), op=OP.subtract,
        )
        nc.sync.dma_start(out2.ap().rearrange("(t p) o -> p t o", p=P), fin[:])
    nc.compile()
    return nc


def _prepare(edge_index):
    key = hash(np.asarray(edge_index).tobytes())
    if key in _CACHE:
        return _CACHE[key]
    NCHK, cores, order = _prep_edges(edge_index)
    l0 = _build_l0()
    l1 = _build_l1(NCHK)
    l2 = _build_l2(NCHK)
    _CACHE.clear()
    _CACHE[key] = (NCHK, cores, order, l0, l1, l2)
    return _CACHE[key]


# c-major permutation: cm column j = c*8+h  <->  original column h*64+c
_CMJ = np.arange(HEADS * HID)
_CM_FROM = (_CMJ % HEADS) * HID + _CMJ // HEADS   # cm[:, j] = orig[:, _CM_FROM[j]]


def _host_consts(W1, a1_src, a1_dst, W2, a2_src, a2_dst):
    W1 = np.asarray(W1, np.float32)
    W2 = np.asarray(W2, np.float32)
    a1_src = np.asarray(a1_src, np.float32)
    a1_dst = np.asarray(a1_dst, np.float32)
    a2_src = np.asarray(a2_src, np.float32).reshape(-1)
    a2_dst = np.asarray(a2_dst, np.float32).reshape(-1)

    W1r = W1.reshape(IN, HEADS, HID)
    wsd = np.concatenate(
        [np.einsum("khc,hc->kh", W1r, a1_src), np.einsum("khc,hc->kh", W1r, a1_dst)],
        axis=1,
    )
    W1cm = W1[:, _CM_FROM]
    wv2s = W2 @ a2_src
    wv2d = W2 @ a2_dst
    W2C = np.concatenate([W2, wv2s[:, None], wv2d[:, None]], axis=1)  # [512,42]
    W2Ccm = W2C[_CM_FROM, :]
    w2cb = np.ascontiguousarray(
        W2Ccm.reshape(4, P, 42).transpose(1, 0, 2).reshape(P, 4 * 42)
    )
    return W1cm, wsd, w2cb


def _launch(prog, maps, trace=False):
    try:
        return run_bass_kernel_spmd(prog, maps, list(range(NCORE)), trace=trace)
    except Exception:
        import time as _time
        _time.sleep(5)
        return run_bass_kernel_spmd(prog, maps, list(range(NCORE)), trace=trace)


def _run(inputs, trace=False):
    x = np.asarray(inputs["x"], np.float32)
    edge_index = inputs["edge_index"]
    NCHK, cores, order, l0, l1, l2 = _prepare(edge_index)
    W1cm, wsd, w2cb = _host_consts(
        inputs["W1"], inputs["a1_src"], inputs["a1_dst"],
        inputs["W2"], inputs["a2_src"], inputs["a2_dst"],
    )
    W1bf = W1cm.astype(_bf16)
    wsdbf = wsd.astype(_bf16)
    w2cbf = w2cb.astype(_bf16)
    idn = np.eye(P, dtype=np.float32).astype(_bf16)

    xpad = np.zeros((NPAD, IN), np.float32)
    xpad[:N] = x

    # ---- L0 ----------------------------------------------------------------
    maps0 = []
    for c in range(NCORE):
        xT = np.ascontiguousarray(xpad[c * SHARD : (c + 1) * SHARD].T.astype(_bf16))
        maps0.append(dict(xT=xT, w1=W1bf, wsd=wsdbf))
    r0 = _launch(l0, maps0, trace)
    hfull = np.concatenate([r0.results[c]["hsh"] for c in range(NCORE)], axis=0)
    esd = np.concatenate([r0.results[c]["esd"] for c in range(NCORE)], axis=0)

    ht = np.zeros((NPAD + 1, HEADS * HID), _bf16)
    ht[:NPAD] = hfull  # already c-major (W1cm) and bf16
    es_ext = np.full((NPAD + 1, HEADS), PADV, np.float32)
    es_ext[:NPAD] = esd[:, :HEADS]
    ed_full = esd[:, HEADS:]

    # ---- L1 ----------------------------------------------------------------
    maps1 = []
    for c in range(NCORE):
        s = cores[c]
        ess = _slotmaj(es_ext[s["srcn"]]).astype(_bf16)
        edtile = np.ascontiguousarray(
            ed_full[s["nodes"]].transpose(1, 0, 2).reshape(P, TILES * HEADS)
        ).astype(_bf16)
        maps1.append(dict(ht=ht, idx=s["idx"], ess=ess, edt=edtile,
                          w2cb=w2cbf, idn=idn))
    r1 = _launch(l1, maps1, trace)

    h2full = np.zeros((NPAD, 42), np.float32)
    for c in range(NCORE):
        nodes = cores[c]["nodes"].reshape(-1)          # slot (t*128+p) -> node
        h2full[nodes] = r1.results[c]["h2r"]
    h2full[N:] = 0.0

    # ---- L2 ----------------------------------------------------------------
    ht2 = np.zeros((NPAD + 1, P), _bf16)
    ht2[:NPAD, :OUT] = h2full[:, :OUT].astype(_bf16)
    e2s_ext = np.full(NPAD + 1, PADV, np.float32)
    e2s_ext[:NPAD] = h2full[:, 40]
    e2d_n = h2full[:, 41]
    maps2 = []
    for c in range(NCORE):
        s = cores[c]
        e2sv = _slotmaj(e2s_ext[s["srcn"]][:, None]).astype(_bf16)
        e2dt = np.ascontiguousarray(
            e2d_n[s["nodes"]].T
        ).astype(np.float32)
        maps2.append(dict(ht2=ht2, idx2=s["idx"], e2s=e2sv, e2dt=e2dt, idn2=idn))
    r2 = _launch(l2, maps2, trace)

    out = np.zeros((NPAD, OUT), np.float32)
    for c in range(NCORE):
        nodes = cores[c]["nodes"].reshape(-1)
        out[nodes] = r2.results[c]["out2"]
    out = out[:N]
    ns = None
    if all(r.exec_time_ns is not None for r in (r0, r1, r2)):
        ns = r0.exec_time_ns + r1.exec_time_ns + r2.exec_time_ns
    return np.ascontiguousarray(out, dtype=np.float32), ns


def kernel(**inputs) -> np.ndarray:
    out, _ = _run(inputs, trace=False)
    return out
